# revision 3
# baseline (speedup 1.0000x reference)
"""Causal multi-head attention (B=2, H=16, S=2048, D=128, fp32) on 8 TRN2
NeuronCores.

Sharding: batch*heads = 32 (b,h) pairs, 4 per core (pure data/head parallel,
no collectives). v3 design (software-pipelined for the in-order engine
queues):

  - Q,K are PE-transposed into [d, s] layout; the transpose batches for
    query-superblock s+1 are emitted at the tail of superblock s so they
    overlap the exp/PV stream instead of stalling it.
  - Scores are computed *transposed* (st[k, q] = K_blk @ Q^T) with float32r
    matmuls (N=512) into mixed PSUM supertiles: a 4-bank [128, 2048] tile
    and a 2-bank [128, 1024] tile alternate (patterns [4] / [4,2,2] /
    [4,2,4,2] / [4,2,4,2,4] key-tiles per superblock), so most ScalarE exps
    cover 2048 columns, amortizing the ~352-cycle ACT overhead.
  - Causal mask: only the diagonal 128x128 sub-block of each diagonal score
    tile gets a NEG mask add; sub-blocks strictly above the diagonal are
    exp'd as garbage but never read (the PV loop skips kb > t).
  - PV runs in natural output layout: out[q, d] += pt_sub[k, q].T @ v[k, d]
    with pt stationary (bf16, FWL) and V natural moving with a ones-column
    appended (N=129): column 128 of each PSUM accumulator collects the
    softmax row-sum for free. PV matmul chains for superblock s-1 are
    interleaved between the score groups of superblock s to keep the PE
    dense (HAM stays at K=8/8) while ScalarE works through the exps.
  - Row-sum reciprocals via DVE over [128, 2] PSUM column slices; normalize
    via tensor_scalar_mul with a [128, 1] scalar AP; fp32 natural-layout
    output DMA'd straight out. No output transposes, no broadcast matmuls,
    no vector tree-adds.
"""

import numpy as np
import ml_dtypes
from contextlib import ExitStack

B, H, S, D = 2, 16, 2048, 128
NCORES = 8
HPC = (B * H) // NCORES  # heads per core
P = 128                  # tile partition size
NQS = 512                # query superblock width
NT = S // P              # 16 key tiles per head
NS = S // NQS            # 4 query superblocks per head
KPS = NQS // P           # 4 key tiles per query superblock
VAUG = 130               # vb_aug row stride (129 used, padded for alignment)
NEG = -1.0e9

# kb-group sizes per superblock: alternate the 4-bank and 2-bank score
# supertiles so exp(g) overlaps the matmuls of g+1.
GROUPS = {0: [4], 1: [4, 2, 2], 2: [4, 2, 4, 2], 3: [4, 2, 4, 2, 4]}

_cache = {}


def _build():
    import concourse.tile as tile
    from concourse import bacc, mybir

    f32 = mybir.dt.float32
    f32r = mybir.dt.float32r
    bf16 = mybir.dt.bfloat16
    Exp = mybir.ActivationFunctionType.Exp

    nc = bacc.Bacc("TRN2", target_bir_lowering=False, debug=False,
                   num_devices=NCORES)
    q_ext = nc.dram_tensor("query", [HPC, S, D], f32, kind="ExternalInput").ap()
    k_ext = nc.dram_tensor("key", [HPC, S, D], f32, kind="ExternalInput").ap()
    v_ext = nc.dram_tensor("value", [HPC, S, D], f32, kind="ExternalInput").ap()
    sb_ext = nc.dram_tensor("scale_b", [P, 1], f32, kind="ExternalInput").ap()
    dm_ext = nc.dram_tensor("diagm", [P, P], f32, kind="ExternalInput").ap()
    id_ext = nc.dram_tensor("ident", [P, P], f32, kind="ExternalInput").ap()
    ng_ext = nc.dram_tensor("negc", [P, 1], f32, kind="ExternalInput").ap()
    out_ext = nc.dram_tensor("out", [HPC, S, D], f32, kind="ExternalOutput").ap()

    with tile.TileContext(nc) as tc, ExitStack() as ctx:
        consts = ctx.enter_context(tc.tile_pool(name="consts", bufs=1))
        sb_t = consts.tile([P, 1], f32, tag="sb")
        nc.sync.dma_start(sb_t[:], sb_ext[:])
        dm_t = consts.tile([P, P], f32, tag="dm")
        nc.sync.dma_start(dm_t[:], dm_ext[:])
        id_t = consts.tile([P, P], f32, tag="id")
        nc.sync.dma_start(id_t[:], id_ext[:])
        ng_t = consts.tile([P, 1], f32, tag="ng")
        nc.sync.dma_start(ng_t[:], ng_ext[:])

        p_nat = ctx.enter_context(tc.tile_pool(name="nat", bufs=2))
        p_tt = ctx.enter_context(tc.tile_pool(name="tt", bufs=2))
        p_pt = ctx.enter_context(tc.tile_pool(name="pt", bufs=1))
        p_osb = ctx.enter_context(tc.tile_pool(name="osb", bufs=2))
        p_rs = ctx.enter_context(tc.tile_pool(name="rs", bufs=2))
        # PSUM: st4 [128,2048] (4 banks) + st2/tp [128,1024] (2) + oa (2) = 8
        p_ps = ctx.enter_context(tc.tile_pool(name="ps", bufs=1, space="PSUM"))

        heads = {}

        def dma_head(h):
            qn = p_nat.tile([P, NT, P], f32, tag="qn", name=f"qn{h}")
            nc.sync.dma_start(qn[:], q_ext[h].rearrange("(t p) d -> p t d", p=P))
            kn = p_nat.tile([P, NT, P], f32, tag="kn", name=f"kn{h}")
            nc.sync.dma_start(kn[:], k_ext[h].rearrange("(t p) d -> p t d", p=P))
            vn = p_nat.tile([P, NT, P], f32, tag="vn", name=f"vn{h}")
            nc.sync.dma_start(vn[:], v_ext[h].rearrange("(t p) d -> p t d", p=P))
            vb = p_tt.tile([P, NT, VAUG], bf16, tag="vb", name=f"vb{h}")
            nc.gpsimd.memset(vb[:, :, P:P + 1], 1.0)
            for c in range(4):
                nc.gpsimd.tensor_copy(vb[:, 4 * c:4 * c + 4, 0:P],
                                      vn[:, 4 * c:4 * c + 4, :])
            qt = p_tt.tile([P, S], f32r, tag="qt", name=f"qt{h}")
            kt = p_tt.tile([P, S], f32r, tag="kt", name=f"kt{h}")
            heads[h] = dict(qn=qn, kn=kn, vn=vn, vb=vb, qt=qt, kt=kt,
                            kbmap={}, oa=None)

        def transposes(h, s):
            """PE-transpose Q chunk s and K chunk s into qt/kt columns."""
            hd = heads[h]
            for nat, tr in ((hd["qn"], hd["qt"]), (hd["kn"], hd["kt"])):
                tp = p_ps.tile([P, NQS], f32, tag="st2", name=f"tp{h}_{s}")
                for jj in range(4):
                    t = 4 * s + jj
                    nc.tensor.transpose(
                        tp[:, jj * P:(jj + 1) * P], nat[:, t, :], id_t[:])
                nc.vector.tensor_copy(tr[:, s * NQS:(s + 1) * NQS], tp[:])

        def score_group(h, s, kb0, gsz):
            """Score MMs + diag mask + exp for kb0..kb0+gsz-1 of superblock s.
            Returns nothing; records pt slices in heads[h]['kbmap']."""
            hd = heads[h]
            tag = "st4" if gsz == 4 else "st2"
            st = p_ps.tile([P, gsz * NQS], f32, tag=tag, name=f"st{h}_{s}_{kb0}")
            for j in range(gsz):
                kb = kb0 + j
                nc.tensor.matmul(
                    st[:, j * NQS:(j + 1) * NQS],
                    hd["kt"][:, kb * P:(kb + 1) * P],
                    hd["qt"][:, s * NQS:(s + 1) * NQS],
                    start=True, stop=True,
                )
                u = kb - KPS * s
                if 0 <= u < KPS:
                    c0 = j * NQS + u * P
                    nc.vector.tensor_add(
                        st[:, c0:c0 + P], st[:, c0:c0 + P], dm_t[:])
            pt = p_pt.tile([P, gsz * NQS], bf16, tag=f"pt{gsz}",
                           bufs=(5 if gsz == 4 else 6), name=f"pt{h}_{s}_{kb0}")
            nc.scalar.activation(pt[:], st[:], Exp, bias=ng_t[:], scale=sb_t[:])
            for j in range(gsz):
                hd["kbmap"][(s, kb0 + j)] = (pt, j * NQS)

        def pv_chunks(h, s):
            """Return a list of closures: one PV accumulation chain per
            q-block of superblock s, then a finalize (recip+mul+dma)."""
            hd = heads[h]

            def start(hd=hd, h=h, s=s):
                hd["oa"] = p_ps.tile([P, 2, NQS], f32, tag="oa",
                                     name=f"oa{h}_{s}")

            def q_block(u, hd=hd, s=s):
                t = KPS * s + u
                oa = hd["oa"]
                dst = oa[:, u // 2,
                         (u % 2) * (P + 1):(u % 2) * (P + 1) + P + 1]
                for kb in range(t + 1):
                    pt, base = hd["kbmap"][(s, kb)]
                    nc.tensor.matmul(
                        dst,
                        pt[:, base + u * P:base + u * P + P],
                        hd["vb"][:, kb, 0:P + 1],
                        start=(kb == 0), stop=(kb == t),
                    )

            def finalize(hd=hd, h=h, s=s):
                oa = hd["oa"]
                rs = p_rs.tile([P, 2, 2], f32, tag="rs", name=f"rs{h}_{s}")
                for m in range(2):
                    nc.vector.reciprocal(
                        rs[:, :, m],
                        oa[:, :, m * (P + 1) + P:m * (P + 1) + P + 1])
                osb = p_osb.tile([P, KPS, P], f32, tag="osb",
                                 name=f"osb{h}_{s}")
                for u in range(KPS):
                    nc.vector.tensor_scalar_mul(
                        osb[:, u, :],
                        oa[:, u // 2, (u % 2) * (P + 1):(u % 2) * (P + 1) + P],
                        rs[:, u // 2, (u % 2):(u % 2) + 1],
                    )
                nc.sync.dma_start(
                    out_ext[h, s * NQS:(s + 1) * NQS, :].rearrange(
                        "(j p) d -> p j d", p=P),
                    osb[:],
                )

            return [start] + [lambda u=u: q_block(u) for u in range(KPS)] \
                + [finalize]

        # flat software-pipelined schedule over (h, s)
        pv_fifo = []
        dma_head(0)
        transposes(0, 0)
        for h in range(HPC):
            if h + 1 < HPC:
                dma_head(h + 1)
            for s in range(NS):
                gplan = GROUPS[s]
                kb0 = 0
                npop = (len(pv_fifo) + len(gplan) - 1) // max(len(gplan), 1)
                for g, gsz in enumerate(gplan):
                    score_group(h, s, kb0, gsz)
                    kb0 += gsz
                    for _ in range(npop):
                        if pv_fifo:
                            pv_fifo.pop(0)()
                # drain any remaining PV work of the previous superblock
                while pv_fifo:
                    pv_fifo.pop(0)()
                # transposes for the next superblock (or next head's s=0)
                if s + 1 < NS:
                    transposes(h, s + 1)
                elif h + 1 < HPC:
                    transposes(h + 1, 0)
                pv_fifo = pv_chunks(h, s)
        while pv_fifo:
            pv_fifo.pop(0)()
    nc.compile()
    return nc


def get_nc():
    if "nc" not in _cache:
        _cache["nc"] = _build()
    return _cache["nc"]


def make_in_maps(query, key, value, scale):
    q = np.ascontiguousarray(np.asarray(query, dtype=np.float32)).reshape(B * H, S, D)
    k = np.ascontiguousarray(np.asarray(key, dtype=np.float32)).reshape(B * H, S, D)
    v = np.ascontiguousarray(np.asarray(value, dtype=np.float32)).reshape(B * H, S, D)
    sc = float(np.asarray(scale).reshape(-1)[0])

    scale_b = np.full((P, 1), sc, dtype=np.float32)
    # diagm[dk, dq] = 0 if dq >= dk else NEG (causal within diagonal block)
    dks = np.arange(P)[:, None]
    dqs = np.arange(P)[None, :]
    diagm = np.where(dqs >= dks, 0.0, NEG).astype(np.float32)
    ident = np.eye(P, dtype=np.float32)
    negc = np.full((P, 1), -50.0, dtype=np.float32)

    in_maps = []
    for c in range(NCORES):
        sl = slice(c * HPC, (c + 1) * HPC)
        in_maps.append({
            "query": np.ascontiguousarray(q[sl]),
            "key": np.ascontiguousarray(k[sl]),
            "value": np.ascontiguousarray(v[sl]),
            "scale_b": scale_b,
            "diagm": diagm,
            "ident": ident,
            "negc": negc,
        })
    return in_maps


def kernel(query, key, value, scale):
    from concourse.bass_utils import run_bass_kernel_spmd

    nc = get_nc()
    in_maps = make_in_maps(query, key, value, scale)
    res = run_bass_kernel_spmd(nc, in_maps, core_ids=list(range(NCORES)))
    out = np.empty((B * H, S, D), dtype=np.float32)
    for c in range(NCORES):
        out[c * HPC:(c + 1) * HPC] = res.results[c]["out"]
    return out.reshape(B, H, S, D)


# revision 4
# speedup vs baseline: 1.4010x; 1.4010x over previous
"""Causal multi-head attention (B=2, H=16, S=2048, D=128, fp32) on 8 TRN2
NeuronCores.

Sharding: batch*heads = 32 (b,h) pairs, 4 per core (pure data/head parallel,
no collectives). v4 design (software-pipelined for the in-order engine
queues):

  - Q,K are PE-transposed into [d, s] layout; the two transpose batches for
    query-superblock s+1 are spread between score groups of superblock s
    (their own 2 PSUM slots), so they never stall the score/exp ping-pong.
  - Scores are computed *transposed* (st[k, q] = K_blk @ Q^T) with float32r
    matmuls (N=512) into double-buffered 2-bank [128, 1024] PSUM supertiles
    (2 key tiles each). One ScalarE exp per supertile; for the diagonal
    pairs the kb order is swapped so the causally-dead prefix sits at the
    start of the ACT span and is trimmed off (fewer exp'd elements).
  - Causal mask: only the diagonal 128x128 sub-block of each diagonal score
    tile gets a NEG mask add; sub-blocks strictly above the diagonal are
    exp'd as garbage (or trimmed) but never read (the PV loop skips kb > t).
  - PV runs in natural output layout: out[q, d] += pt_sub[k, q].T @ v[k, d]
    with pt stationary (bf16, FWL) and V natural moving with a ones-column
    appended (N=129): column 128 of each PSUM accumulator collects the
    softmax row-sum for free. PV chains for superblock s-1 are split into
    <=5-matmul pieces and interleaved between the score groups of s, so the
    PE stays dense (HAM at K=8/8) without head-of-line-blocking the score
    matmuls that feed ScalarE.
  - Row-sum reciprocals via DVE over [128, 2] PSUM column slices; normalize
    via tensor_scalar_mul with a [128, 1] scalar AP; fp32 natural-layout
    output DMA'd straight out. No output transposes, no broadcast matmuls,
    no vector tree-adds, no 1-partition reciprocals.
"""

import numpy as np
import ml_dtypes
from contextlib import ExitStack

B, H, S, D = 2, 16, 2048, 128
NCORES = 8
HPC = (B * H) // NCORES  # heads per core
P = 128                  # tile partition size
NQS = 512                # query superblock width
NT = S // P              # 16 key tiles per head
NS = S // NQS            # 4 query superblocks per head
KPS = NQS // P           # 4 key tiles per query superblock
VAUG = 130               # vb_aug row stride (129 used, padded for alignment)
PVCH = 5                 # max matmuls per interleaved PV piece
NEG = -1.0e9

_cache = {}


def _build():
    import concourse.tile as tile
    from concourse import bacc, mybir

    f32 = mybir.dt.float32
    f32r = mybir.dt.float32r
    bf16 = mybir.dt.bfloat16
    Exp = mybir.ActivationFunctionType.Exp

    nc = bacc.Bacc("TRN2", target_bir_lowering=False, debug=False,
                   num_devices=NCORES)
    q_ext = nc.dram_tensor("query", [HPC, S, D], f32, kind="ExternalInput").ap()
    k_ext = nc.dram_tensor("key", [HPC, S, D], f32, kind="ExternalInput").ap()
    v_ext = nc.dram_tensor("value", [HPC, S, D], f32, kind="ExternalInput").ap()
    sb_ext = nc.dram_tensor("scale_b", [P, 1], f32, kind="ExternalInput").ap()
    dm_ext = nc.dram_tensor("diagm", [P, P], f32, kind="ExternalInput").ap()
    id_ext = nc.dram_tensor("ident", [P, P], f32, kind="ExternalInput").ap()
    ng_ext = nc.dram_tensor("negc", [P, 1], f32, kind="ExternalInput").ap()
    out_ext = nc.dram_tensor("out", [HPC, S, D], f32, kind="ExternalOutput").ap()

    with tile.TileContext(nc) as tc, ExitStack() as ctx:
        consts = ctx.enter_context(tc.tile_pool(name="consts", bufs=1))
        sb_t = consts.tile([P, 1], f32, tag="sb")
        nc.sync.dma_start(sb_t[:], sb_ext[:])
        dm_t = consts.tile([P, P], f32, tag="dm")
        nc.sync.dma_start(dm_t[:], dm_ext[:])
        id_t = consts.tile([P, P], f32, tag="id")
        nc.sync.dma_start(id_t[:], id_ext[:])
        ng_t = consts.tile([P, 1], f32, tag="ng")
        nc.sync.dma_start(ng_t[:], ng_ext[:])

        p_nat = ctx.enter_context(tc.tile_pool(name="nat", bufs=2))
        p_tt = ctx.enter_context(tc.tile_pool(name="tt", bufs=2))
        p_pt = ctx.enter_context(tc.tile_pool(name="pt", bufs=12))
        p_osb = ctx.enter_context(tc.tile_pool(name="osb", bufs=2))
        p_rs = ctx.enter_context(tc.tile_pool(name="rs", bufs=2))
        # PSUM: st 2x[128,1024](4 banks) + tp 2x[128,512](2) + oa 1x2banks = 8
        p_ps = ctx.enter_context(tc.tile_pool(name="ps", bufs=1, space="PSUM"))

        heads = {}

        def dma_head(h):
            qn = p_nat.tile([P, NT, P], f32, tag="qn", name=f"qn{h}")
            nc.sync.dma_start(qn[:], q_ext[h].rearrange("(t p) d -> p t d", p=P))
            kn = p_nat.tile([P, NT, P], f32, tag="kn", name=f"kn{h}")
            nc.sync.dma_start(kn[:], k_ext[h].rearrange("(t p) d -> p t d", p=P))
            vn = p_nat.tile([P, NT, P], f32, tag="vn", name=f"vn{h}")
            nc.sync.dma_start(vn[:], v_ext[h].rearrange("(t p) d -> p t d", p=P))
            vb = p_tt.tile([P, NT, VAUG], bf16, tag="vb", name=f"vb{h}")
            nc.gpsimd.memset(vb[:, :, P:P + 1], 1.0)
            for c in range(4):
                nc.gpsimd.tensor_copy(vb[:, 4 * c:4 * c + 4, 0:P],
                                      vn[:, 4 * c:4 * c + 4, :])
            qt = p_tt.tile([P, S], f32r, tag="qt", name=f"qt{h}")
            kt = p_tt.tile([P, S], f32r, tag="kt", name=f"kt{h}")
            heads[h] = dict(qn=qn, kn=kn, vn=vn, vb=vb, qt=qt, kt=kt,
                            kbmap={}, oa=None)

        def transpose_batch(h, s, which):
            """PE-transpose one of Q/K chunk s into qt/kt columns."""
            hd = heads[h]
            nat, tr = ((hd["qn"], hd["qt"]) if which == 0
                       else (hd["kn"], hd["kt"]))
            tp = p_ps.tile([P, NQS], f32, tag="tp", bufs=2,
                           name=f"tp{h}_{s}_{which}")
            for jj in range(4):
                t = 4 * s + jj
                nc.tensor.transpose(
                    tp[:, jj * P:(jj + 1) * P], nat[:, t, :], id_t[:])
            nc.vector.tensor_copy(tr[:, s * NQS:(s + 1) * NQS], tp[:])

        def score_group(h, s, pair):
            """Score MMs + diag mask + exp for a kb pair of superblock s."""
            hd = heads[h]
            st = p_ps.tile([P, 2 * NQS], f32, tag="st", bufs=2,
                           name=f"st{h}_{s}_{pair[0]}")
            trim = 0
            for j, kb in enumerate(pair):
                nc.tensor.matmul(
                    st[:, j * NQS:(j + 1) * NQS],
                    hd["kt"][:, kb * P:(kb + 1) * P],
                    hd["qt"][:, s * NQS:(s + 1) * NQS],
                    start=True, stop=True,
                )
                u = kb - KPS * s
                if 0 <= u < KPS:
                    c0 = j * NQS + u * P
                    nc.vector.tensor_add(
                        st[:, c0:c0 + P], st[:, c0:c0 + P], dm_t[:])
                    if j == 0:
                        # leading tile: cols < u*P are causally dead -> trim
                        trim = u * P
            pt = p_pt.tile([P, 2 * NQS], bf16, tag="pt",
                           name=f"pt{h}_{s}_{pair[0]}")
            nc.scalar.activation(pt[:, trim:], st[:, trim:], Exp,
                                 bias=ng_t[:], scale=sb_t[:])
            for j, kb in enumerate(pair):
                hd["kbmap"][(s, kb)] = (pt, j * NQS)

        def pv_pieces(h, s):
            """Closures: oa alloc, <=PVCH-matmul accumulation pieces per
            q-block of superblock s, then finalize (recip+mul+dma)."""
            hd = heads[h]

            def start(hd=hd, h=h, s=s):
                hd["oa"] = p_ps.tile([P, 2, NQS], f32, tag="oa",
                                     name=f"oa{h}_{s}")

            def piece(u, lo, hi, hd=hd, s=s):
                t = KPS * s + u
                oa = hd["oa"]
                dst = oa[:, u // 2,
                         (u % 2) * (P + 1):(u % 2) * (P + 1) + P + 1]
                for kb in range(lo, hi):
                    pt, base = hd["kbmap"][(s, kb)]
                    nc.tensor.matmul(
                        dst,
                        pt[:, base + u * P:base + u * P + P],
                        hd["vb"][:, kb, 0:P + 1],
                        start=(kb == 0), stop=(kb == t),
                    )

            def finalize(hd=hd, h=h, s=s):
                oa = hd["oa"]
                rs = p_rs.tile([P, 2, 2], f32, tag="rs", name=f"rs{h}_{s}")
                for m in range(2):
                    nc.vector.reciprocal(
                        rs[:, :, m],
                        oa[:, :, m * (P + 1) + P:m * (P + 1) + P + 1])
                osb = p_osb.tile([P, KPS, P], f32, tag="osb",
                                 name=f"osb{h}_{s}")
                for u in range(KPS):
                    nc.vector.tensor_scalar_mul(
                        osb[:, u, :],
                        oa[:, u // 2, (u % 2) * (P + 1):(u % 2) * (P + 1) + P],
                        rs[:, u // 2, (u % 2):(u % 2) + 1],
                    )
                nc.sync.dma_start(
                    out_ext[h, s * NQS:(s + 1) * NQS, :].rearrange(
                        "(j p) d -> p j d", p=P),
                    osb[:],
                )

            pieces = [start]
            for u in range(KPS):
                t = KPS * s + u
                for lo in range(0, t + 1, PVCH):
                    pieces.append(lambda u=u, lo=lo, hi=min(lo + PVCH, t + 1):
                                  piece(u, lo, hi))
            pieces.append(finalize)
            return pieces

        def kb_pairs(s):
            """kb pairs for superblock s; diagonal pairs ordered descending
            so the dead prefix is trimmable from the exp."""
            pairs = [(kb, kb + 1) for kb in range(0, 4 * s, 2)]
            pairs += [(KPS * s + 1, KPS * s), (KPS * s + 3, KPS * s + 2)]
            return pairs

        # flat software-pipelined schedule over (h, s)
        pv_fifo = []
        dma_head(0)
        transpose_batch(0, 0, 0)
        transpose_batch(0, 0, 1)
        for h in range(HPC):
            if h + 1 < HPC:
                dma_head(h + 1)
            for s in range(NS):
                pairs = kb_pairs(s)
                # transposes for the next superblock (or next head's s=0)
                tposes = ([(h, s + 1)] if s + 1 < NS
                          else ([(h + 1, 0)] if h + 1 < HPC else []))
                for g, pair in enumerate(pairs):
                    score_group(h, s, pair)
                    npop = ((len(pv_fifo) + len(pairs) - g - 1)
                            // (len(pairs) - g))
                    for _ in range(npop):
                        pv_fifo.pop(0)()
                    if g == 0 and tposes:
                        transpose_batch(tposes[0][0], tposes[0][1], 0)
                    if g == 1 and tposes:
                        transpose_batch(tposes[0][0], tposes[0][1], 1)
                while pv_fifo:
                    pv_fifo.pop(0)()
                pv_fifo = pv_pieces(h, s)
        while pv_fifo:
            pv_fifo.pop(0)()
    nc.compile()
    return nc


def get_nc():
    if "nc" not in _cache:
        _cache["nc"] = _build()
    return _cache["nc"]


def make_in_maps(query, key, value, scale):
    q = np.ascontiguousarray(np.asarray(query, dtype=np.float32)).reshape(B * H, S, D)
    k = np.ascontiguousarray(np.asarray(key, dtype=np.float32)).reshape(B * H, S, D)
    v = np.ascontiguousarray(np.asarray(value, dtype=np.float32)).reshape(B * H, S, D)
    sc = float(np.asarray(scale).reshape(-1)[0])

    scale_b = np.full((P, 1), sc, dtype=np.float32)
    # diagm[dk, dq] = 0 if dq >= dk else NEG (causal within diagonal block)
    dks = np.arange(P)[:, None]
    dqs = np.arange(P)[None, :]
    diagm = np.where(dqs >= dks, 0.0, NEG).astype(np.float32)
    ident = np.eye(P, dtype=np.float32)
    negc = np.full((P, 1), -50.0, dtype=np.float32)

    in_maps = []
    for c in range(NCORES):
        sl = slice(c * HPC, (c + 1) * HPC)
        in_maps.append({
            "query": np.ascontiguousarray(q[sl]),
            "key": np.ascontiguousarray(k[sl]),
            "value": np.ascontiguousarray(v[sl]),
            "scale_b": scale_b,
            "diagm": diagm,
            "ident": ident,
            "negc": negc,
        })
    return in_maps


def kernel(query, key, value, scale):
    from concourse.bass_utils import run_bass_kernel_spmd

    nc = get_nc()
    in_maps = make_in_maps(query, key, value, scale)
    res = run_bass_kernel_spmd(nc, in_maps, core_ids=list(range(NCORES)))
    out = np.empty((B * H, S, D), dtype=np.float32)
    for c in range(NCORES):
        out[c * HPC:(c + 1) * HPC] = res.results[c]["out"]
    return out.reshape(B, H, S, D)


# revision 9
# speedup vs baseline: 1.4723x; 1.0509x over previous
"""Causal multi-head attention (B=2, H=16, S=2048, D=128, fp32) on 8 TRN2
NeuronCores.

Sharding: batch*heads = 32 (b,h) pairs, 4 per core (pure data/head parallel,
no collectives). v4 design (software-pipelined for the in-order engine
queues):

  - Q,K are PE-transposed into [d, s] layout; the two transpose batches for
    query-superblock s+1 are spread between score groups of superblock s
    (their own 2 PSUM slots), so they never stall the score/exp ping-pong.
  - Scores are computed *transposed* (st[k, q] = K_blk @ Q^T) with float32r
    matmuls (N=512) into double-buffered 2-bank [128, 1024] PSUM supertiles
    (2 key tiles each). One ScalarE exp per supertile; for the diagonal
    pairs the kb order is swapped so the causally-dead prefix sits at the
    start of the ACT span and is trimmed off (fewer exp'd elements).
  - Causal mask: only the diagonal 128x128 sub-block of each diagonal score
    tile gets a NEG mask add; sub-blocks strictly above the diagonal are
    exp'd as garbage (or trimmed) but never read (the PV loop skips kb > t).
  - PV runs in natural output layout: out[q, d] += pt_sub[k, q].T @ v[k, d]
    with pt stationary (bf16, FWL) and V natural moving with a ones-column
    appended (N=129): column 128 of each PSUM accumulator collects the
    softmax row-sum for free. PV chains for superblock s-1 are split into
    <=5-matmul pieces and interleaved between the score groups of s, so the
    PE stays dense (HAM at K=8/8) without head-of-line-blocking the score
    matmuls that feed ScalarE.
  - Row-sum reciprocals via DVE over [128, 2] PSUM column slices; normalize
    via tensor_scalar_mul with a [128, 1] scalar AP; fp32 natural-layout
    output DMA'd straight out. No output transposes, no broadcast matmuls,
    no vector tree-adds, no 1-partition reciprocals.
"""

import numpy as np
import ml_dtypes
from contextlib import ExitStack

B, H, S, D = 2, 16, 2048, 128
NCORES = 8
HPC = (B * H) // NCORES  # heads per core
P = 128                  # tile partition size
NQS = 512                # query superblock width
NT = S // P              # 16 key tiles per head
NS = S // NQS            # 4 query superblocks per head
KPS = NQS // P           # 4 key tiles per query superblock
VAUG = 130               # vb_aug row stride (129 used, padded for alignment)
PVCH = 5                 # max matmuls per interleaved PV piece
NEG = -1.0e9

_cache = {}


def _build():
    import concourse.tile as tile
    from concourse import bacc, mybir

    f32 = mybir.dt.float32
    f32r = mybir.dt.float32r
    bf16 = mybir.dt.bfloat16
    Exp = mybir.ActivationFunctionType.Exp

    nc = bacc.Bacc("TRN2", target_bir_lowering=False, debug=False,
                   num_devices=NCORES)
    q_ext = nc.dram_tensor("query", [HPC, S, D], f32, kind="ExternalInput").ap()
    k_ext = nc.dram_tensor("key", [HPC, S, D], f32, kind="ExternalInput").ap()
    v_ext = nc.dram_tensor("value", [HPC, S, D], f32, kind="ExternalInput").ap()
    sb_ext = nc.dram_tensor("scale_b", [P, 1], f32, kind="ExternalInput").ap()
    dm_ext = nc.dram_tensor("diagm", [P, P], f32, kind="ExternalInput").ap()
    id_ext = nc.dram_tensor("ident", [P, P], f32, kind="ExternalInput").ap()
    ng_ext = nc.dram_tensor("negc", [P, 1], f32, kind="ExternalInput").ap()
    out_ext = nc.dram_tensor("out", [HPC, S, D], f32, kind="ExternalOutput").ap()

    with tile.TileContext(nc) as tc, ExitStack() as ctx:
        consts = ctx.enter_context(tc.tile_pool(name="consts", bufs=1))
        sb_t = consts.tile([P, 1], f32, tag="sb")
        nc.sync.dma_start(sb_t[:], sb_ext[:])
        dm_t = consts.tile([P, P], f32, tag="dm")
        nc.sync.dma_start(dm_t[:], dm_ext[:])
        id_t = consts.tile([P, P], f32, tag="id")
        nc.sync.dma_start(id_t[:], id_ext[:])
        ng_t = consts.tile([P, 1], f32, tag="ng")
        nc.sync.dma_start(ng_t[:], ng_ext[:])

        p_nat = ctx.enter_context(tc.tile_pool(name="nat", bufs=2))
        p_tt = ctx.enter_context(tc.tile_pool(name="tt", bufs=2))
        p_pt = ctx.enter_context(tc.tile_pool(name="pt", bufs=12))
        p_osb = ctx.enter_context(tc.tile_pool(name="osb", bufs=2))
        p_rs = ctx.enter_context(tc.tile_pool(name="rs", bufs=2))
        # PSUM: st 2x[128,1024](4 banks) + tp 2x[128,512](2) + oa 1x2banks = 8
        p_ps = ctx.enter_context(tc.tile_pool(name="ps", bufs=1, space="PSUM"))

        heads = {}

        def dma_head(h):
            qn = p_nat.tile([P, NT, P], f32, tag="qn", name=f"qn{h}")
            kn = p_nat.tile([P, NT, P], f32, tag="kn", name=f"kn{h}")
            for nat, ext in ((qn, q_ext), (kn, k_ext)):
                for c in range(4):
                    nc.sync.dma_start(
                        nat[:, 4 * c:4 * c + 4, :],
                        ext[h, c * NQS:(c + 1) * NQS, :].rearrange(
                            "(t p) d -> p t d", p=P))
            vn = p_nat.tile([P, NT, P], f32, tag="vn", name=f"vn{h}")
            nc.sync.dma_start(vn[:], v_ext[h].rearrange("(t p) d -> p t d", p=P))
            vb = p_tt.tile([P, NT, VAUG], bf16, tag="vb", name=f"vb{h}")
            nc.gpsimd.memset(vb[:, :, P:P + 1], 1.0)
            for c in range(4):
                nc.gpsimd.tensor_copy(vb[:, 4 * c:4 * c + 4, 0:P],
                                      vn[:, 4 * c:4 * c + 4, :])
            qt = p_tt.tile([P, S], f32r, tag="qt", name=f"qt{h}")
            kt = p_tt.tile([P, S], f32r, tag="kt", name=f"kt{h}")
            heads[h] = dict(qn=qn, kn=kn, vn=vn, vb=vb, qt=qt, kt=kt,
                            kbmap={}, oa=None)

        def transpose_batch(h, s):
            """PE-transpose Q and K chunk s into qt/kt columns, borrowing one
            st slot (Q batch in cols 0-511, K batch in 512-1023)."""
            hd = heads[h]
            tp = p_ps.tile([P, 2 * NQS], f32, tag="st", bufs=3,
                           name=f"tp{h}_{s}")
            for w, (nat, tr) in enumerate(((hd["qn"], hd["qt"]),
                                           (hd["kn"], hd["kt"]))):
                for jj in range(4):
                    t = 4 * s + jj
                    nc.tensor.transpose(
                        tp[:, w * NQS + jj * P:w * NQS + (jj + 1) * P],
                        nat[:, t, :], id_t[:])
                nc.vector.tensor_copy(tr[:, s * NQS:(s + 1) * NQS],
                                      tp[:, w * NQS:(w + 1) * NQS])

        def score_group(h, s, pair):
            """Score MMs + diag mask + exp for a kb pair of superblock s."""
            hd = heads[h]
            st = p_ps.tile([P, 2 * NQS], f32, tag="st", bufs=3,
                           name=f"st{h}_{s}_{pair[0]}")
            trim = 0
            for j, kb in enumerate(pair):
                nc.tensor.matmul(
                    st[:, j * NQS:(j + 1) * NQS],
                    hd["kt"][:, kb * P:(kb + 1) * P],
                    hd["qt"][:, s * NQS:(s + 1) * NQS],
                    start=True, stop=True,
                )
                u = kb - KPS * s
                if 0 <= u < KPS:
                    c0 = j * NQS + u * P
                    nc.vector.tensor_add(
                        st[:, c0:c0 + P], st[:, c0:c0 + P], dm_t[:])
                    if j == 0:
                        # leading tile: cols < u*P are causally dead -> trim
                        trim = u * P
            pt = p_pt.tile([P, 2 * NQS], bf16, tag="pt",
                           name=f"pt{h}_{s}_{pair[0]}")
            nc.scalar.activation(pt[:, trim:], st[:, trim:], Exp,
                                 bias=ng_t[:], scale=sb_t[:])
            for j, kb in enumerate(pair):
                hd["kbmap"][(s, kb)] = (pt, j * NQS)

        def pv_pieces(h, s):
            """Closures: oa alloc, <=PVCH-matmul accumulation pieces per
            q-block of superblock s, then finalize (recip+mul+dma)."""
            hd = heads[h]

            def start(hd=hd, h=h, s=s):
                hd["oa"] = p_ps.tile([P, 2, NQS], f32, tag="oa",
                                     name=f"oa{h}_{s}")

            def piece(u, lo, hi, hd=hd, s=s):
                t = KPS * s + u
                oa = hd["oa"]
                dst = oa[:, u // 2,
                         (u % 2) * (P + 1):(u % 2) * (P + 1) + P + 1]
                for kb in range(lo, hi):
                    pt, base = hd["kbmap"][(s, kb)]
                    nc.tensor.matmul(
                        dst,
                        pt[:, base + u * P:base + u * P + P],
                        hd["vb"][:, kb, 0:P + 1],
                        start=(kb == 0), stop=(kb == t),
                    )

            def finalize(hd=hd, h=h, s=s):
                oa = hd["oa"]
                rs = p_rs.tile([P, 2, 2], f32, tag="rs", name=f"rs{h}_{s}")
                for m in range(2):
                    nc.vector.reciprocal(
                        rs[:, :, m],
                        oa[:, :, m * (P + 1) + P:m * (P + 1) + P + 1])
                osb = p_osb.tile([P, KPS, P], f32, tag="osb",
                                 name=f"osb{h}_{s}")
                for u in range(KPS):
                    nc.vector.tensor_scalar_mul(
                        osb[:, u, :],
                        oa[:, u // 2, (u % 2) * (P + 1):(u % 2) * (P + 1) + P],
                        rs[:, u // 2, (u % 2):(u % 2) + 1],
                    )
                nc.sync.dma_start(
                    out_ext[h, s * NQS:(s + 1) * NQS, :].rearrange(
                        "(j p) d -> p j d", p=P),
                    osb[:],
                )

            pieces = [start]
            for u in range(KPS):
                t = KPS * s + u
                for lo in range(0, t + 1, PVCH):
                    pieces.append(lambda u=u, lo=lo, hi=min(lo + PVCH, t + 1):
                                  piece(u, lo, hi))
            pieces.append(finalize)
            return pieces

        def kb_pairs(s):
            """kb pairs for superblock s; diagonal pairs ordered descending
            so the dead prefix is trimmable from the exp."""
            pairs = [(kb, kb + 1) for kb in range(0, 4 * s, 2)]
            pairs += [(KPS * s + 1, KPS * s), (KPS * s + 3, KPS * s + 2)]
            return pairs

        # flat software-pipelined schedule over (h, s)
        pv_fifo = []
        dma_head(0)
        transpose_batch(0, 0)
        for h in range(HPC):
            if h + 1 < HPC:
                dma_head(h + 1)
            for s in range(NS):
                pairs = kb_pairs(s)
                # transposes for the next superblock (or next head's s=0)
                tposes = ([(h, s + 1)] if s + 1 < NS
                          else ([(h + 1, 0)] if h + 1 < HPC else []))
                for g, pair in enumerate(pairs):
                    score_group(h, s, pair)
                    npop = ((len(pv_fifo) + len(pairs) - g - 1)
                            // (len(pairs) - g))
                    for _ in range(npop):
                        pv_fifo.pop(0)()
                    if g == 0 and tposes:
                        transpose_batch(*tposes[0])
                while pv_fifo:
                    pv_fifo.pop(0)()
                pv_fifo = pv_pieces(h, s)
        while pv_fifo:
            pv_fifo.pop(0)()
    nc.compile()
    return nc


def get_nc():
    if "nc" not in _cache:
        _cache["nc"] = _build()
    return _cache["nc"]


def make_in_maps(query, key, value, scale):
    q = np.ascontiguousarray(np.asarray(query, dtype=np.float32)).reshape(B * H, S, D)
    k = np.ascontiguousarray(np.asarray(key, dtype=np.float32)).reshape(B * H, S, D)
    v = np.ascontiguousarray(np.asarray(value, dtype=np.float32)).reshape(B * H, S, D)
    sc = float(np.asarray(scale).reshape(-1)[0])

    scale_b = np.full((P, 1), sc, dtype=np.float32)
    # diagm[dk, dq] = 0 if dq >= dk else NEG (causal within diagonal block)
    dks = np.arange(P)[:, None]
    dqs = np.arange(P)[None, :]
    diagm = np.where(dqs >= dks, 0.0, NEG).astype(np.float32)
    ident = np.eye(P, dtype=np.float32)
    negc = np.full((P, 1), -50.0, dtype=np.float32)

    in_maps = []
    for c in range(NCORES):
        sl = slice(c * HPC, (c + 1) * HPC)
        in_maps.append({
            "query": np.ascontiguousarray(q[sl]),
            "key": np.ascontiguousarray(k[sl]),
            "value": np.ascontiguousarray(v[sl]),
            "scale_b": scale_b,
            "diagm": diagm,
            "ident": ident,
            "negc": negc,
        })
    return in_maps


def kernel(query, key, value, scale):
    from concourse.bass_utils import run_bass_kernel_spmd

    nc = get_nc()
    in_maps = make_in_maps(query, key, value, scale)
    res = run_bass_kernel_spmd(nc, in_maps, core_ids=list(range(NCORES)))
    out = np.empty((B * H, S, D), dtype=np.float32)
    for c in range(NCORES):
        out[c * HPC:(c + 1) * HPC] = res.results[c]["out"]
    return out.reshape(B, H, S, D)
